# revision 2
# baseline (speedup 1.0000x reference)
"""GCN message-passing layer (4x GCNConv sum) on 8 Trainium2 NeuronCores.

out[d] = sum_i ( segment_sum_{e in E_i, dst=d} x[src_e] ) @ W_i

Self-contained kernel: takes FULL inputs, shards destination nodes across the
8 cores (graph parallel, x replicated), runs one SPMD Bass/Tile program via
run_bass_kernel_spmd, returns the FULL [N, H] output.

Per-core pipeline (no collectives needed):
  - Host groups edges by (core, set, dst-tile(256), src-bank(32768)); each
    group is padded to a cross-core-uniform multiple of 128 so one program
    fits every core's data (pad: src=0, dstf=-1).
  - dma_gather (int16 bank-local idxs, single_packet=False) stages x[src]
    rows (bf16) into SBUF.
  - Each 128-edge chunk is segment-summed into PSUM by TensorE with a one-hot
    S[e, d] = (dstf[e] == d) built by VectorE is_equal against an iota row;
    pad edges select no column.  gpsum[t] holds g_i^T = [128h, 256d] for the
    4 edge sets (one 2-bank PSUM tile).
  - Phase 2: out[d, :] += (g_i^T).T @ W_i in fp32, accumulated over the 4
    sets in PSUM, then DMA'd to the per-core output rows.
"""
import math
import sys

sys.path.insert(0, "/opt/trn_rl_repo")

import numpy as np
import ml_dtypes

from concourse import bass, mybir, tile, bacc
from concourse.bass_utils import run_bass_kernel_spmd

P = 128
N_CORES = 8
DT_TILE = 256
BANK_ROWS = 32768
SPAN = 2
N_SETS = 4


class _Cfg:
    def __init__(self, n_nodes):
        self.n_nodes = n_nodes
        self.npc = n_nodes // N_CORES
        self.nt = math.ceil(self.npc / DT_TILE)
        self.n_banks = math.ceil(n_nodes / BANK_ROWS)
        self.x_rows_pad = self.n_banks * BANK_ROWS
        self.n_spans = math.ceil(self.nt / SPAN)


def _host_prep(cfg, edges_list):
    NC, NS, NT, NB = N_CORES, N_SETS, cfg.nt, cfg.n_banks
    counts = np.zeros((NC, NS, NT, NB), np.int64)
    per_set = []
    for i, e in enumerate(edges_list):
        src = np.asarray(e[0], np.int64)
        dst = np.asarray(e[1], np.int64)
        core = dst // cfg.npc
        dloc = dst % cfg.npc
        t = dloc // DT_TILE
        b = src // BANK_ROWS
        key = (core * NT + t) * NB + b
        counts[:, i] = np.bincount(key, minlength=NC * NT * NB).reshape(NC, NT, NB)
        order = np.argsort(key, kind="stable")
        per_set.append((src[order], dloc[order], key[order]))

    C = -(-counts.max(axis=0) // P)

    col_of = np.zeros((NS, NT, NB), np.int64)
    unit_cols = {}
    col = 0
    for s in range(cfg.n_spans):
        ts = range(s * SPAN, min((s + 1) * SPAN, NT))
        for i in range(NS):
            for b in range(NB):
                for t in ts:
                    col_of[i, t, b] = col
                    col += C[i, t, b]
                unit_cols[(s, i, b)] = int(C[i, list(ts), b].sum())
    totc = col
    tot_slots = totc * P

    idx_mats, dstf_mats = [], []
    for c in range(NC):
        slot_src = np.zeros(tot_slots, np.int64)
        slot_dstf = np.full(tot_slots, -1.0, np.float32)
        for i in range(NS):
            src_s, dloc_s, key_s = per_set[i]
            lo = np.searchsorted(key_s, c * NT * NB)
            hi = np.searchsorted(key_s, (c + 1) * NT * NB)
            src_c, dloc_c, key_c = src_s[lo:hi], dloc_s[lo:hi], key_s[lo:hi]
            t_c = (key_c // NB) % NT
            b_c = key_c % NB
            gstart = np.searchsorted(key_c, key_c)
            rank = np.arange(len(key_c)) - gstart
            slot = col_of[i, t_c, b_c] * P + rank
            slot_src[slot] = src_c - b_c * BANK_ROWS
            slot_dstf[slot] = (dloc_c - t_c * DT_TILE).astype(np.float32)
        idx16 = slot_src.reshape(tot_slots // 16, 16).T.astype(np.int16)
        idx_mats.append(np.tile(idx16, (8, 1)))
        dstf_mats.append(slot_dstf.reshape(totc, P).T.copy())

    return dict(C=C, col_of=col_of, unit_cols=unit_cols, totc=totc,
                idx_mats=idx_mats, dstf_mats=dstf_mats)


def _first_b(C, i, t):
    for b in range(C.shape[2]):
        if C[i, t, b] > 0:
            return b
    return -1


def _last_b(C, i, t):
    for b in range(C.shape[2] - 1, -1, -1):
        if C[i, t, b] > 0:
            return b
    return -1


def _build_kernel(cfg, prep):
    NS, NT, NB = N_SETS, cfg.nt, cfg.n_banks
    C, col_of, unit_cols, totc = (prep["C"], prep["col_of"], prep["unit_cols"],
                                  prep["totc"])
    DT = DT_TILE
    msg_dt = mybir.dt.bfloat16
    s_dt = msg_dt

    nc = bacc.Bacc("TRN2", target_bir_lowering=False, debug=False,
                   num_devices=N_CORES)
    x = nc.dram_tensor("x", [cfg.x_rows_pad, P], msg_dt, kind="ExternalInput").ap()
    idx_d = nc.dram_tensor("idx", [P, totc * 8], mybir.dt.int16, kind="ExternalInput").ap()
    dstf_d = nc.dram_tensor("dstf", [P, totc], mybir.dt.float32, kind="ExternalInput").ap()
    iota_d = nc.dram_tensor("iota", [P, DT], s_dt, kind="ExternalInput").ap()
    w_d = nc.dram_tensor("w", [NS * P, P], mybir.dt.float32, kind="ExternalInput").ap()
    out_d = nc.dram_tensor("out", [cfg.npc, P], mybir.dt.float32, kind="ExternalOutput").ap()

    with tile.TileContext(nc) as tc:
        with tc.tile_pool(name="const", bufs=1) as constp, \
             tc.tile_pool(name="idxp", bufs=3) as idxp, \
             tc.tile_pool(name="dstfp", bufs=3) as dstfp, \
             tc.tile_pool(name="stg", bufs=2) as stgp, \
             tc.tile_pool(name="sp", bufs=6) as spool, \
             tc.tile_pool(name="gsb", bufs=3) as gsbp, \
             tc.tile_pool(name="osb", bufs=4) as osbp, \
             tc.tile_pool(name="gps", bufs=2, space="PSUM") as gpsp, \
             tc.tile_pool(name="ops", bufs=2, space="PSUM") as opsp:

            iota_sb = constp.tile([P, DT], s_dt)
            nc.sync.dma_start(out=iota_sb[:], in_=iota_d[:])
            w_sb = constp.tile([P, NS, P], mybir.dt.float32)
            for i in range(NS):
                nc.sync.dma_start(out=w_sb[:, i, :], in_=w_d[i * P:(i + 1) * P, :])

            for s in range(cfg.n_spans):
                ts = list(range(s * SPAN, min((s + 1) * SPAN, NT)))
                gpsum = {}
                for t in ts:
                    gpsum[t] = gpsp.tile([P, NS, DT], mybir.dt.float32,
                                         space="PSUM", tag="gp", name="gp")
                for i in range(NS):
                    i_col0 = int(col_of[i, ts[0], 0])
                    i_cols = sum(unit_cols[(s, i, b)] for b in range(NB))
                    if i_cols == 0:
                        continue
                    idx_sb = idxp.tile([P, i_cols * 8], mybir.dt.int16, tag="idx", name="idx")
                    nc.sync.dma_start(out=idx_sb[:], in_=idx_d[:, i_col0 * 8:(i_col0 + i_cols) * 8])
                    dstf_sb = dstfp.tile([P, i_cols], mybir.dt.float32, tag="dstf", name="dstf")
                    nc.sync.dma_start(out=dstf_sb[:], in_=dstf_d[:, i_col0:i_col0 + i_cols])

                    stg = {}
                    for b in range(NB):
                        ncols = unit_cols[(s, i, b)]
                        if ncols == 0:
                            continue
                        stg[b] = stgp.tile([P, ncols, P], msg_dt, tag=f"stg{b}", name=f"stg{b}")
                        u_col0 = int(col_of[i, ts[0], b])
                        for c0 in range(0, ncols, 48):
                            cw = min(48, ncols - c0)
                            rel16 = (u_col0 - i_col0 + c0) * 8
                            n_idx = cw * P
                            nc.gpsimd.dma_gather(
                                out_ap=stg[b][:, c0:c0 + cw, :],
                                in_ap=x[b * BANK_ROWS:(b + 1) * BANK_ROWS, :],
                                idxs_ap=idx_sb[:, rel16:rel16 + cw * 8],
                                num_idxs=n_idx,
                                num_idxs_reg=n_idx,
                                elem_size=P,
                                single_packet=False,
                            )
                    for b in range(NB):
                        if b not in stg:
                            continue
                        for t in ts:
                            cc = int(C[i, t, b])
                            if cc == 0:
                                continue
                            gcol0 = int(col_of[i, t, b])
                            rel_s = gcol0 - i_col0
                            rel_b = gcol0 - int(col_of[i, ts[0], b])
                            for k in range(cc):
                                s_tile = spool.tile([P, DT], s_dt, tag="s", name="s")
                                nc.vector.tensor_scalar(
                                    out=s_tile[:], in0=iota_sb[:],
                                    scalar1=dstf_sb[:, rel_s + k:rel_s + k + 1],
                                    scalar2=None, op0=mybir.AluOpType.is_equal)
                                first = (b == _first_b(C, i, t)) and k == 0
                                last = (b == _last_b(C, i, t)) and k == cc - 1
                                nc.tensor.matmul(
                                    out=gpsum[t][:, i, :],
                                    lhsT=stg[b][:, rel_b + k, :],
                                    rhs=s_tile[:],
                                    start=first, stop=last)
                for t in ts:
                    gsb = gsbp.tile([P, NS, DT], mybir.dt.float32, tag="g", name="g")
                    for i in range(NS):
                        if C[i, t].sum() == 0:
                            nc.vector.memset(gsb[:, i, :], 0.0)
                        else:
                            nc.vector.tensor_copy(out=gsb[:, i, :], in_=gpsum[t][:, i, :])
                    for half in range(DT // P):
                        d0 = t * DT + half * P
                        rows = min(P, cfg.npc - d0)
                        if rows <= 0:
                            continue
                        opsum = opsp.tile([P, P], mybir.dt.float32, space="PSUM",
                                          tag="o", name="o")
                        for i in range(NS):
                            nc.tensor.matmul(
                                out=opsum[:],
                                lhsT=gsb[:, i, half * P:(half + 1) * P],
                                rhs=w_sb[:, i, :],
                                start=(i == 0), stop=(i == NS - 1))
                        ot = osbp.tile([P, P], mybir.dt.float32, tag="ot", name="ot")
                        nc.vector.tensor_copy(out=ot[:], in_=opsum[:])
                        nc.sync.dma_start(out=out_d[d0:d0 + rows, :], in_=ot[:rows, :])
    nc.compile()
    return nc


def _make_in_maps(cfg, prep, x, w_list):
    n_nodes = x.shape[0]
    x_pad = np.zeros((cfg.x_rows_pad, P), np.float32)
    x_pad[:n_nodes] = np.asarray(x, np.float32)
    x_pad = x_pad.astype(ml_dtypes.bfloat16)
    iota = np.tile(np.arange(DT_TILE, dtype=np.float32)[None, :], (P, 1)).astype(
        ml_dtypes.bfloat16)
    w_cat = np.concatenate([np.asarray(w, np.float32) for w in w_list], axis=0)

    return [{
        "x": x_pad,
        "idx": prep["idx_mats"][c],
        "dstf": prep["dstf_mats"][c],
        "iota": iota,
        "w": w_cat,
    } for c in range(N_CORES)]


def kernel(hidden_states, edges_i, edges_ii, edges_iii, edges_a,
           W_i, W_ii, W_iii, W_a):
    x = np.asarray(hidden_states, np.float32)
    cfg = _Cfg(x.shape[0])
    edges_list = [np.asarray(e) for e in (edges_i, edges_ii, edges_iii, edges_a)]
    w_list = [W_i, W_ii, W_iii, W_a]

    prep = _host_prep(cfg, edges_list)
    nc = _build_kernel(cfg, prep)
    in_maps = _make_in_maps(cfg, prep, x, w_list)

    res = run_bass_kernel_spmd(nc, in_maps, core_ids=list(range(N_CORES)))
    out = np.concatenate([res.results[c]["out"] for c in range(N_CORES)], axis=0)
    return out.astype(np.float32)



# revision 6
# speedup vs baseline: 10.0105x; 10.0105x over previous
"""GCN message-passing layer (4x GCNConv sum) on 8 Trainium2 NeuronCores.

out[d] = sum_i ( segment_sum_{e in E_i, dst=d} x[src_e] ) @ W_i

Self-contained kernel: takes FULL inputs, shards destination nodes across the
8 cores (graph parallel, x replicated), runs one SPMD Bass/Tile program via
run_bass_kernel_spmd, returns the FULL [N, H] output.

Per-core pipeline (no collectives needed):
  - Host groups edges by (core, set, dst-tile(256), src-bank(32768)); each
    group is padded to a cross-core-uniform multiple of 128 so one program
    fits every core's data (pad: src=0, dstf=-1).
  - dma_gather (int16 bank-local idxs, single_packet=False) stages x[src]
    rows (bf16) into SBUF.
  - Each 128-edge chunk is segment-summed into PSUM by TensorE with a one-hot
    S[e, d] = (dstf[e] == d) built by VectorE is_equal against an iota row;
    pad edges select no column.  gpsum[t] holds g_i^T = [128h, 256d] for the
    4 edge sets (one 2-bank PSUM tile).
  - Phase 2: out[d, :] += (g_i^T).T @ W_i in fp32, accumulated over the 4
    sets in PSUM, then DMA'd to the per-core output rows.
"""
import math
import sys

sys.path.insert(0, "/opt/trn_rl_repo")

import numpy as np
import ml_dtypes

from concourse import bass, mybir, tile, bacc
from concourse.bass_utils import run_bass_kernel_spmd

P = 128
N_CORES = 8
DT_TILE = 256
BANK_ROWS = 32768
SPAN = 2
N_SETS = 4


class _Cfg:
    def __init__(self, n_nodes):
        self.n_nodes = n_nodes
        self.npc = n_nodes // N_CORES
        self.nt = math.ceil(self.npc / DT_TILE)
        self.n_banks = math.ceil(n_nodes / BANK_ROWS)
        self.x_rows_pad = self.n_banks * BANK_ROWS
        self.n_spans = math.ceil(self.nt / SPAN)


def _host_prep(cfg, edges_list):
    NC, NS, NT, NB = N_CORES, N_SETS, cfg.nt, cfg.n_banks
    counts = np.zeros((NC, NS, NT, NB), np.int64)
    per_set = []
    for i, e in enumerate(edges_list):
        src = np.asarray(e[0], np.int64)
        dst = np.asarray(e[1], np.int64)
        core = dst // cfg.npc
        dloc = dst % cfg.npc
        t = dloc // DT_TILE
        b = src // BANK_ROWS
        key = (core * NT + t) * NB + b
        counts[:, i] = np.bincount(key, minlength=NC * NT * NB).reshape(NC, NT, NB)
        order = np.argsort(key, kind="stable")
        per_set.append((src[order], dloc[order], key[order]))

    C = -(-counts.max(axis=0) // P)

    col_of = np.zeros((NS, NT, NB), np.int64)
    unit_cols = {}
    col = 0
    for s in range(cfg.n_spans):
        ts = range(s * SPAN, min((s + 1) * SPAN, NT))
        for i in range(NS):
            for b in range(NB):
                for t in ts:
                    col_of[i, t, b] = col
                    col += C[i, t, b]
                unit_cols[(s, i, b)] = int(C[i, list(ts), b].sum())
    totc = col
    tot_slots = totc * P

    idx_mats, dstf_mats = [], []
    for c in range(NC):
        slot_src = np.zeros(tot_slots, np.int64)
        slot_dstf = np.full(tot_slots, -1.0, np.float32)
        for i in range(NS):
            src_s, dloc_s, key_s = per_set[i]
            lo = np.searchsorted(key_s, c * NT * NB)
            hi = np.searchsorted(key_s, (c + 1) * NT * NB)
            src_c, dloc_c, key_c = src_s[lo:hi], dloc_s[lo:hi], key_s[lo:hi]
            t_c = (key_c // NB) % NT
            b_c = key_c % NB
            gstart = np.searchsorted(key_c, key_c)
            rank = np.arange(len(key_c)) - gstart
            slot = col_of[i, t_c, b_c] * P + rank
            slot_src[slot] = src_c - b_c * BANK_ROWS
            slot_dstf[slot] = (dloc_c - t_c * DT_TILE).astype(np.float32)
        idx16 = slot_src.reshape(tot_slots // 16, 16).T.astype(np.int16)
        idx_mats.append(np.tile(idx16, (8, 1)))
        dstf_mats.append(slot_dstf.reshape(totc, P).T.copy())

    return dict(C=C, col_of=col_of, unit_cols=unit_cols, totc=totc,
                idx_mats=idx_mats, dstf_mats=dstf_mats)


def _first_b(C, i, t):
    for b in range(C.shape[2]):
        if C[i, t, b] > 0:
            return b
    return -1


def _last_b(C, i, t):
    for b in range(C.shape[2] - 1, -1, -1):
        if C[i, t, b] > 0:
            return b
    return -1


def _build_kernel(cfg, prep, reps=1):
    NS, NT, NB = N_SETS, cfg.nt, cfg.n_banks
    C, col_of, unit_cols, totc = (prep["C"], prep["col_of"], prep["unit_cols"],
                                  prep["totc"])
    DT = DT_TILE
    msg_dt = mybir.dt.bfloat16
    s_dt = msg_dt

    nc = bacc.Bacc("TRN2", target_bir_lowering=False, debug=False,
                   num_devices=N_CORES, num_swdge_queues=4)
    gq = [0]  # round-robin queue counter for dma_gather
    x = nc.dram_tensor("x", [cfg.x_rows_pad, P], msg_dt, kind="ExternalInput").ap()
    idx_d = nc.dram_tensor("idx", [P, totc * 8], mybir.dt.int16, kind="ExternalInput").ap()
    dstf_d = nc.dram_tensor("dstf", [P, totc], mybir.dt.float32, kind="ExternalInput").ap()
    iota_d = nc.dram_tensor("iota", [P, DT], s_dt, kind="ExternalInput").ap()
    w_d = nc.dram_tensor("w", [NS * P, P], mybir.dt.float32, kind="ExternalInput").ap()
    out_d = nc.dram_tensor("out", [cfg.npc, P], mybir.dt.float32, kind="ExternalOutput").ap()

    with tile.TileContext(nc) as tc:
        with tc.tile_pool(name="const", bufs=1) as constp, \
             tc.tile_pool(name="idxp", bufs=3) as idxp, \
             tc.tile_pool(name="dstfp", bufs=3) as dstfp, \
             tc.tile_pool(name="stg", bufs=2) as stgp, \
             tc.tile_pool(name="sp", bufs=6) as spool, \
             tc.tile_pool(name="gsb", bufs=3) as gsbp, \
             tc.tile_pool(name="osb", bufs=4) as osbp, \
             tc.tile_pool(name="gps", bufs=2, space="PSUM") as gpsp, \
             tc.tile_pool(name="ops", bufs=2, space="PSUM") as opsp:

            iota_sb = constp.tile([P, DT], s_dt)
            nc.sync.dma_start(out=iota_sb[:], in_=iota_d[:])
            w_sb = constp.tile([P, NS, P], mybir.dt.float32)
            for i in range(NS):
                nc.sync.dma_start(out=w_sb[:, i, :], in_=w_d[i * P:(i + 1) * P, :])

            for s in range(cfg.n_spans * reps):
                s = s % cfg.n_spans
                ts = list(range(s * SPAN, min((s + 1) * SPAN, NT)))
                gpsum = {}
                for t in ts:
                    gpsum[t] = gpsp.tile([P, NS, DT], mybir.dt.float32,
                                         space="PSUM", tag="gp", name="gp")
                for i in range(NS):
                    i_col0 = int(col_of[i, ts[0], 0])
                    i_cols = sum(unit_cols[(s, i, b)] for b in range(NB))
                    if i_cols == 0:
                        continue
                    idx_sb = idxp.tile([P, i_cols * 8], mybir.dt.int16, tag="idx", name="idx")
                    nc.sync.dma_start(out=idx_sb[:], in_=idx_d[:, i_col0 * 8:(i_col0 + i_cols) * 8])
                    dstf_sb = dstfp.tile([P, i_cols], mybir.dt.float32, tag="dstf", name="dstf")
                    nc.sync.dma_start(out=dstf_sb[:], in_=dstf_d[:, i_col0:i_col0 + i_cols])

                    stg = {}
                    for b in range(NB):
                        ncols = unit_cols[(s, i, b)]
                        if ncols == 0:
                            continue
                        stg[b] = stgp.tile([P, ncols, P], msg_dt, tag=f"stg{b}", name=f"stg{b}")
                        u_col0 = int(col_of[i, ts[0], b])
                        for c0 in range(0, ncols, 48):
                            cw = min(48, ncols - c0)
                            rel16 = (u_col0 - i_col0 + c0) * 8
                            n_idx = cw * P
                            nc.gpsimd.dma_gather(
                                out_ap=stg[b][:, c0:c0 + cw, :],
                                in_ap=x[b * BANK_ROWS:(b + 1) * BANK_ROWS, :],
                                idxs_ap=idx_sb[:, rel16:rel16 + cw * 8],
                                num_idxs=n_idx,
                                num_idxs_reg=n_idx,
                                elem_size=P,
                                single_packet=False,
                                queue_num=gq[0] % 4,
                            )
                            gq[0] += 1
                    for b in range(NB):
                        if b not in stg:
                            continue
                        for t in ts:
                            cc = int(C[i, t, b])
                            if cc == 0:
                                continue
                            gcol0 = int(col_of[i, t, b])
                            rel_s = gcol0 - i_col0
                            rel_b = gcol0 - int(col_of[i, ts[0], b])
                            for k in range(cc):
                                s_tile = spool.tile([P, DT], s_dt, tag="s", name="s")
                                nc.vector.tensor_scalar(
                                    out=s_tile[:], in0=iota_sb[:],
                                    scalar1=dstf_sb[:, rel_s + k:rel_s + k + 1],
                                    scalar2=None, op0=mybir.AluOpType.is_equal)
                                first = (b == _first_b(C, i, t)) and k == 0
                                last = (b == _last_b(C, i, t)) and k == cc - 1
                                nc.tensor.matmul(
                                    out=gpsum[t][:, i, :],
                                    lhsT=stg[b][:, rel_b + k, :],
                                    rhs=s_tile[:],
                                    start=first, stop=last)
                for t in ts:
                    gsb = gsbp.tile([P, NS, DT], mybir.dt.float32, tag="g", name="g")
                    for i in range(NS):
                        if C[i, t].sum() == 0:
                            nc.vector.memset(gsb[:, i, :], 0.0)
                        else:
                            nc.vector.tensor_copy(out=gsb[:, i, :], in_=gpsum[t][:, i, :])
                    for half in range(DT // P):
                        d0 = t * DT + half * P
                        rows = min(P, cfg.npc - d0)
                        if rows <= 0:
                            continue
                        opsum = opsp.tile([P, P], mybir.dt.float32, space="PSUM",
                                          tag="o", name="o")
                        for i in range(NS):
                            nc.tensor.matmul(
                                out=opsum[:],
                                lhsT=gsb[:, i, half * P:(half + 1) * P],
                                rhs=w_sb[:, i, :],
                                start=(i == 0), stop=(i == NS - 1))
                        ot = osbp.tile([P, P], mybir.dt.float32, tag="ot", name="ot")
                        nc.vector.tensor_copy(out=ot[:], in_=opsum[:])
                        nc.sync.dma_start(out=out_d[d0:d0 + rows, :], in_=ot[:rows, :])
    nc.compile()
    return nc


def _make_in_maps(cfg, prep, x, w_list):
    n_nodes = x.shape[0]
    x_pad = np.zeros((cfg.x_rows_pad, P), np.float32)
    x_pad[:n_nodes] = np.asarray(x, np.float32)
    x_pad = x_pad.astype(ml_dtypes.bfloat16)
    iota = np.tile(np.arange(DT_TILE, dtype=np.float32)[None, :], (P, 1)).astype(
        ml_dtypes.bfloat16)
    w_cat = np.concatenate([np.asarray(w, np.float32) for w in w_list], axis=0)

    return [{
        "x": x_pad,
        "idx": prep["idx_mats"][c],
        "dstf": prep["dstf_mats"][c],
        "iota": iota,
        "w": w_cat,
    } for c in range(N_CORES)]


def kernel(hidden_states, edges_i, edges_ii, edges_iii, edges_a,
           W_i, W_ii, W_iii, W_a):
    x = np.asarray(hidden_states, np.float32)
    cfg = _Cfg(x.shape[0])
    edges_list = [np.asarray(e) for e in (edges_i, edges_ii, edges_iii, edges_a)]
    w_list = [W_i, W_ii, W_iii, W_a]

    prep = _host_prep(cfg, edges_list)
    nc = _build_kernel(cfg, prep)
    in_maps = _make_in_maps(cfg, prep, x, w_list)

    res = run_bass_kernel_spmd(nc, in_maps, core_ids=list(range(N_CORES)))
    out = np.concatenate([res.results[c]["out"] for c in range(N_CORES)], axis=0)
    return out.astype(np.float32)



# revision 35
# speedup vs baseline: 14.2924x; 1.4277x over previous
"""GCN message-passing layer (4x GCNConv sum) on 8 Trainium2 NeuronCores.

out[d] = sum_i ( segment_sum_{e in E_i, dst=d} x[src_e] ) @ W_i

Raw-block SPMD kernel (no Tile scheduler): destination nodes sharded across
8 cores, x replicated. Per core:
  - Host groups edges by (core, set, dst-tile(128), src-bank(32768)), sorts
    each group by src (DRAM locality), pads groups to a cross-core-uniform
    multiple of 128 (pad: src=0, dstf=-1).
  - gpsimd dma_gather (int16 bank-local idxs) stages x[src] rows (bf16) into
    SBUF, round-robin over 4 SWDGE queues (4x descriptor throughput).
  - Processing is set-major within each 4-tile span: while PE consumes set
    i's staging, gpsimd prefetches sets i+1, i+2 (stg ring of 3).
  - DVE builds one-hot S[e,d] = (dstf[e]==d) per 128-edge chunk; TensorE
    accumulates g_i^T = stg^T @ S into per-tile PSUM (1 bank, 6-slot ring).
  - ACT copies tile PSUM->SBUF; TensorE phase 2: out_tile = sum_i gsb_i^T.T
    @ W_i into opsum; ACT copies opsum->SBUF; SP DMAs out rows.
All cross-engine sync is explicit semaphores; every core runs an identical
program (counts are cross-core uniform by construction).
"""
import math
import sys
from contextlib import ExitStack

sys.path.insert(0, "/opt/trn_rl_repo")

import numpy as np
import ml_dtypes

from concourse import bass, mybir, bacc
from concourse.bass_utils import run_bass_kernel_spmd
from concourse.library_config import mlp

P = 128
N_CORES = 8
DT = 128           # dst-tile width (one PSUM bank per (tile, 4 sets))
BANK_ROWS = 32768
NS = 4
GSPAN = 4          # tiles per idx/gather span
CALL_COLS = 96     # max 128-edge chunks per dma_gather call (12288 idxs)
SRING = 16         # one-hot ring slots
STGRING = 3        # stg (span,set) ring slots
GPSLOTS = 6        # gpsum PSUM rotation (1 bank each)
OTSLOTS = 4        # opsum/ot rotation


class _Cfg:
    def __init__(self, n_nodes):
        self.n_nodes = n_nodes
        self.npc = n_nodes // N_CORES
        self.nt = math.ceil(self.npc / DT)
        self.n_banks = math.ceil(n_nodes / BANK_ROWS)
        self.x_rows_pad = self.n_banks * BANK_ROWS
        self.n_spans = math.ceil(self.nt / GSPAN)


def _host_prep(cfg, edges_list):
    NC, NT, NB = N_CORES, cfg.nt, cfg.n_banks
    counts = np.zeros((NC, NS, NT, NB), np.int64)
    per_set = []
    for i, e in enumerate(edges_list):
        src = np.asarray(e[0], np.int64)
        dst = np.asarray(e[1], np.int64)
        core = dst // cfg.npc
        dloc = dst % cfg.npc
        t = dloc // DT
        b = src // BANK_ROWS
        key = (core * NT + t) * NB + b
        counts[:, i] = np.bincount(key, minlength=NC * NT * NB).reshape(NC, NT, NB)
        order = np.lexsort((src, key))  # src-sorted within group: DRAM locality
        per_set.append((src[order], dloc[order], key[order]))

    C = -(-counts.max(axis=0) // P)
    # guarantee no (set, tile) is entirely empty (phase-2 reads its psum)
    for i in range(NS):
        for t in range(NT):
            if C[i, t].sum() == 0:
                C[i, t, 0] = 1

    col_of = np.zeros((NS, NT, NB), np.int64)
    unit_cols = {}
    span_col0, span_cols = [], []
    col = 0
    for s in range(cfg.n_spans):
        ts = range(s * GSPAN, min((s + 1) * GSPAN, NT))
        span_col0.append(col)
        for i in range(NS):
            for b in range(NB):
                for t in ts:
                    col_of[i, t, b] = col
                    col += C[i, t, b]
                unit_cols[(s, i, b)] = int(C[i, list(ts), b].sum())
        span_cols.append(col - span_col0[-1])
    totc = col
    tot_slots = totc * P

    idx_mats, dstf_mats = [], []
    for c in range(NC):
        slot_src = np.zeros(tot_slots, np.int64)
        slot_dstf = np.full(tot_slots, -1.0, np.float32)
        for i in range(NS):
            src_s, dloc_s, key_s = per_set[i]
            lo = np.searchsorted(key_s, c * NT * NB)
            hi = np.searchsorted(key_s, (c + 1) * NT * NB)
            src_c, dloc_c, key_c = src_s[lo:hi], dloc_s[lo:hi], key_s[lo:hi]
            t_c = (key_c // NB) % NT
            b_c = key_c % NB
            gstart = np.searchsorted(key_c, key_c)
            rank = np.arange(len(key_c)) - gstart
            slot = col_of[i, t_c, b_c] * P + rank
            slot_src[slot] = src_c - b_c * BANK_ROWS
            slot_dstf[slot] = (dloc_c - t_c * DT).astype(np.float32)
        idx16 = slot_src.reshape(tot_slots // 16, 16).T.astype(np.int16)
        idx_mats.append(np.tile(idx16, (8, 1)))
        dstf_mats.append(slot_dstf.reshape(totc, P).T.copy())

    # gather call list: per (s,i,b), split into <=CALL_COLS chunks
    calls = []
    for s in range(cfg.n_spans):
        for i in range(NS):
            for b in range(NB):
                cols = unit_cols[(s, i, b)]
                if cols == 0:
                    continue
                g0 = int(col_of[i, s * GSPAN, b])
                for c0 in range(0, cols, CALL_COLS):
                    cw = min(CALL_COLS, cols - c0)
                    calls.append(dict(s=s, i=i, b=b, col0=g0 + c0, cols=cw))
    # queue assignment (round-robin) + per-queue cumulative completion counts
    qcnt = [0, 0, 0, 0]
    group_qwaits = {}   # (s,i,b) -> list[(q, count)] to wait for group ready
    span_qwaits = {}    # s -> list[(q, count)] through end of span
    for ci, cl in enumerate(calls):
        q = ci % 4
        cl["q"] = q
        qcnt[q] += 1
        group_qwaits[(cl["s"], cl["i"], cl["b"])] = None
        cl["qcnt"] = qcnt[q]
    # group ready = all its calls done; calls of a group are consecutive
    run_q = [0, 0, 0, 0]
    cur_group = None
    for cl in calls:
        run_q[cl["q"]] = cl["qcnt"]
        g = (cl["s"], cl["i"], cl["b"])
        group_qwaits[g] = [(q, run_q[q]) for q in range(4) if run_q[q] > 0]
        span_qwaits[cl["s"]] = [(q, run_q[q]) for q in range(4) if run_q[q] > 0]
    # minimal per-group waits: only queues used by the group's own calls,
    # at that group's max count (earlier groups' counts are implied by
    # program order of waits on PE)
    gq = {}
    for cl in calls:
        g = (cl["s"], cl["i"], cl["b"])
        gq.setdefault(g, {})[cl["q"]] = cl["qcnt"]
    group_qwaits = {g: sorted(d.items()) for g, d in gq.items()}

    return dict(C=C, col_of=col_of, unit_cols=unit_cols, totc=totc,
                span_col0=span_col0, span_cols=span_cols,
                idx_mats=idx_mats, dstf_mats=dstf_mats,
                calls=calls, group_qwaits=group_qwaits,
                span_qwaits=span_qwaits, qtot=list(qcnt))


def _schedule(cfg, prep):
    """Static per-core schedule: ordered chunk list (set-major within span)
    + per-tile completion chunk counts."""
    C, col_of = prep["C"], prep["col_of"]
    NT, NB = cfg.nt, cfg.n_banks
    chunks = []
    tile_chunk_end = {}   # tile t -> chunk count (1-based) at its completion
    chunks_thru_span = {}
    for s in range(cfg.n_spans):
        ts = range(s * GSPAN, min((s + 1) * GSPAN, NT))
        for i in range(NS):
            for t in ts:
                nib = [(b, int(C[i, t, b])) for b in range(NB) if C[i, t, b] > 0]
                tot_k = sum(n for _, n in nib)
                kk = 0
                for b, cc in nib:
                    for k in range(cc):
                        chunks.append(dict(
                            s=s, t=t, i=i, b=b,
                            col=int(col_of[i, t, b]) + k,
                            start=(kk == 0), stop=(kk == tot_k - 1),
                            first_of_group=(k == 0),
                            last_of_pair=False,
                        ))
                        kk += 1
                if i == NS - 1:
                    tile_chunk_end[t] = len(chunks)
        chunks_thru_span[s] = len(chunks)
    seen = {}
    for ci, ch in enumerate(chunks):
        seen[(ch["s"], ch["i"])] = ci
    for (s, i), ci in seen.items():
        chunks[ci]["last_of_pair"] = True
    return chunks, tile_chunk_end, chunks_thru_span


def _build_kernel(cfg, prep, reps=1):
    NT, NB = cfg.nt, cfg.n_banks
    calls = prep["calls"]
    group_qwaits = prep["group_qwaits"]
    span_qwaits = prep["span_qwaits"]
    qtot = prep["qtot"]
    span_col0, span_cols = prep["span_col0"], prep["span_cols"]
    unit_cols = prep["unit_cols"]
    col_of = prep["col_of"]
    totc = prep["totc"]
    chunks, tile_chunk_end, chunks_thru_span = _schedule(cfg, prep)

    n_chunks = len(chunks)
    n_calls = len(calls)
    n_spans = cfg.n_spans
    max_span_cols = max(span_cols)
    max_i_cols = max(sum(unit_cols[(s, i, b)] for b in range(NB))
                     for s in range(n_spans) for i in range(NS))

    pair_idx = {(s, i): s * NS + i for s in range(n_spans) for i in range(NS)}
    i_col0 = {(s, i): int(col_of[i, s * GSPAN, 0])
              for s in range(n_spans) for i in range(NS)}
    out_rows = {t: max(0, min(DT, cfg.npc - t * DT)) for t in range(NT)}
    n_valid_tiles = sum(1 for t in range(NT) if out_rows[t] > 0)

    msg_dt = mybir.dt.bfloat16

    nc = bacc.Bacc("TRN2", target_bir_lowering=False, debug=False,
                   num_devices=N_CORES, num_swdge_queues=4)
    x = nc.dram_tensor("x", [cfg.x_rows_pad, P], msg_dt, kind="ExternalInput").ap()
    idx_d = nc.dram_tensor("idx", [P, totc * 8], mybir.dt.int16,
                           kind="ExternalInput").ap()
    dstf_d = nc.dram_tensor("dstf", [P, totc], mybir.dt.float32,
                            kind="ExternalInput").ap()
    iota_d = nc.dram_tensor("iota", [P, DT], msg_dt, kind="ExternalInput").ap()
    w_d = nc.dram_tensor("w", [NS * P, P], mybir.dt.float32,
                         kind="ExternalInput").ap()
    out_d = nc.dram_tensor("out", [cfg.npc, P], mybir.dt.float32,
                           kind="ExternalOutput").ap()

    with (
        nc.Block() as block,
        nc.sbuf_tensor("iota_sb", [P, DT], msg_dt) as iota_sb,
        nc.sbuf_tensor("w_sb", [P, NS, P], mybir.dt.float32) as w_sb,
        nc.sbuf_tensor("idxb", [P, 2, max_span_cols * 8], mybir.dt.int16) as idxb,
        nc.sbuf_tensor("dstfb", [P, 2, max_span_cols], mybir.dt.float32) as dstfb,
        nc.sbuf_tensor("stg", [P, STGRING, max_i_cols, P], msg_dt) as stg,
        nc.sbuf_tensor("sring", [P, SRING, DT], msg_dt) as sring,
        nc.sbuf_tensor("gsb", [P, 2, NS, DT], mybir.dt.float32) as gsb,
        nc.sbuf_tensor("ot", [P, OTSLOTS, P], mybir.dt.float32) as ot,
        nc.psum_tensor("gp", [P, GPSLOTS, NS, DT], mybir.dt.float32) as gp,
        nc.psum_tensor("op", [P, OTSLOTS, P], mybir.dt.float32) as op,
        ExitStack() as _stack,
    ):
        _sem = lambda n: _stack.enter_context(nc.semaphore(n))
        cio = _sem("cio")    # const loads done (SP)
        sin = _sem("sin")    # span idx+dstf DMA done (SP, +32/span)
        sgq = [_sem(f"sgq{q}") for q in range(4)]  # gather done per queue
        sstg = _sem("sstg")  # stg (s,i) consumed by PE (+1)
        ss = _sem("ss")      # one-hot ready (DVE, +1/chunk)
        ssf = _sem("ssf")    # one-hot consumed (PE, +1/chunk)
        st = _sem("st")      # tile psum complete (PE, +1/tile)
        sgp = _sem("sgp")    # gsb ready / gpsum freed (ACT, +1/tile)
        sp2 = _sem("sp2")    # opsum ready (PE, +1/tile)
        sot = _sem("sot")    # ot ready (ACT, +1/tile)
        sof = _sem("sof")    # out DMA done (SP, +16/tile)

        @block.sync
        def _(sy: bass.BassEngine):
            sy.dma_start(iota_sb[:], iota_d[:]).then_inc(cio, 16)
            for i in range(NS):
                sy.dma_start(w_sb[:, i, :],
                             w_d[i * P:(i + 1) * P, :]).then_inc(cio, 16)
            tile_seq = 0
            od = 0  # completed-out-DMA self-wait counter

            def emit_out(t):
                nonlocal tile_seq, od
                rows = out_rows[t]
                if rows <= 0:
                    tile_seq += 1
                    return
                sy.wait_ge(sot, tile_seq + 1)
                if od > 0:
                    sy.wait_ge(sof, 16 * od)  # updater-order: prior outs done
                sy.dma_start(out_d[t * DT:t * DT + rows, :],
                             ot[:rows, tile_seq % OTSLOTS, :]).then_inc(sof, 16)
                od += 1
                tile_seq += 1

            for rep in range(reps):
                for s in range(n_spans):
                    gs = rep * n_spans + s
                    if gs >= 1:
                        sy.wait_ge(sin, 32 * gs)  # updater-order: prior ins done
                    if gs >= 2:
                        ps = gs - 2
                        for q, cnt in span_qwaits[ps % n_spans]:
                            sy.wait_ge(sgq[q], 16 * ((ps // n_spans) * qtot[q]
                                                     + cnt))
                        sy.wait_ge(ss, (ps // n_spans) * n_chunks
                                   + chunks_thru_span[ps % n_spans])
                    c0, cw = span_col0[s], span_cols[s]
                    sy.dma_start(idxb[:, gs % 2, 0:cw * 8],
                                 idx_d[:, c0 * 8:(c0 + cw) * 8]).then_inc(sin, 16)
                    sy.wait_ge(sin, 32 * gs + 16)
                    sy.dma_start(dstfb[:, gs % 2, 0:cw],
                                 dstf_d[:, c0:c0 + cw]).then_inc(sin, 16)
                    if gs >= 2:
                        ps_s = (gs - 2) % n_spans
                        for t in range(ps_s * GSPAN,
                                       min((ps_s + 1) * GSPAN, NT)):
                            emit_out(t)
            for gs_tr in range(max(0, reps * n_spans - 2), reps * n_spans):
                s = gs_tr % n_spans
                for t in range(s * GSPAN, min((s + 1) * GSPAN, NT)):
                    emit_out(t)
            for q in range(4):
                if qtot[q] > 0:
                    sy.wait_ge(sgq[q], 16 * reps * qtot[q])
            sy.wait_ge(sof, 16 * n_valid_tiles * reps)

        @block.gpsimd
        def _(g: bass.BassGpSimd):
            g.load_library(mlp)
            g.wait_ge(cio, 16 * (1 + NS))
            ci = 0
            cur_span = -1
            waited_pair = -1
            for rep in range(reps):
                for cl in calls:
                    s, i, b = cl["s"], cl["i"], cl["b"]
                    gs = rep * n_spans + s
                    gpair = rep * n_spans * NS + pair_idx[(s, i)]
                    if gs != cur_span:
                        g.wait_ge(sin, 32 * (gs + 1))
                        cur_span = gs
                    if gpair >= STGRING and gpair != waited_pair:
                        g.wait_ge(sstg, gpair - STGRING + 1)
                        waited_pair = gpair
                    k_q = rep * qtot[cl["q"]] + cl["qcnt"]
                    if k_q > 1:
                        g.wait_ge(sgq[cl["q"]], 16 * (k_q - 1))
                    rel = cl["col0"] - i_col0[(s, i)]
                    srel = cl["col0"] - span_col0[s]
                    n_idx = cl["cols"] * P
                    g.dma_gather(
                        out_ap=stg[:, gpair % STGRING, rel:rel + cl["cols"], :],
                        in_ap=x[b * BANK_ROWS:(b + 1) * BANK_ROWS, :],
                        idxs_ap=idxb[:, gs % 2, srel * 8:(srel + cl["cols"]) * 8],
                        num_idxs=n_idx,
                        num_idxs_reg=n_idx,
                        elem_size=P,
                        single_packet=False,
                        queue_num=cl["q"],
                    ).then_inc(sgq[cl["q"]], 16)
                    ci += 1

        @block.vector
        def _(v: bass.BassVectorEngine):
            v.wait_ge(cio, 16 * (1 + NS))
            idx = 0
            cur_span = -1
            for rep in range(reps):
                for ch in chunks:
                    gs = rep * n_spans + ch["s"]
                    if gs != cur_span:
                        v.wait_ge(sin, 32 * (gs + 1))
                        cur_span = gs
                    if idx >= SRING:
                        v.wait_ge(ssf, idx - SRING + 1)
                    scol = ch["col"] - span_col0[ch["s"]]
                    v.tensor_scalar(
                        out=sring[:, idx % SRING, :], in0=iota_sb[:],
                        scalar1=dstfb[:, gs % 2, scol:scol + 1],
                        scalar2=None, op0=mybir.AluOpType.is_equal,
                    ).then_inc(ss, 1)
                    idx += 1

        @block.tensor
        def _(t_: bass.BassTensorEngine):
            t_.wait_ge(cio, 16 * (1 + NS))
            idx = 0
            tile_seq = 0
            pend = []  # tile_seqs awaiting phase2 (depth 2 for ACT slack)

            def phase2(tseq):
                t_.wait_ge(sgp, tseq + 1)
                if tseq + 1 > OTSLOTS:
                    t_.wait_ge(sot, tseq + 1 - OTSLOTS)
                for i in range(NS):
                    mm = t_.matmul(
                        out=op[:, tseq % OTSLOTS, :],
                        lhsT=gsb[:, tseq % 2, i, :],
                        rhs=w_sb[:, i, :],
                        start=(i == 0), stop=(i == NS - 1))
                    if i == NS - 1:
                        mm.then_inc(sp2, 1)

            for rep in range(reps):
                last_ti = None
                for ch in chunks:
                    s, t, i, b = ch["s"], ch["t"], ch["i"], ch["b"]
                    # first chunk of tile t overall (i == 0 pass)
                    if i == 0 and (t, rep) != last_ti and ch["start"]:
                        if len(pend) >= 2:
                            phase2(pend.pop(0))
                        ts_new = rep * NT + t
                        if ts_new >= GPSLOTS:
                            t_.wait_ge(sgp, ts_new - GPSLOTS + 1)
                        last_ti = (t, rep)
                    if ch["first_of_group"]:
                        for q, cnt in group_qwaits[(s, i, b)]:
                            t_.wait_ge(sgq[q], 16 * (rep * qtot[q] + cnt))
                    t_.wait_ge(ss, idx + 1)
                    gpair = rep * n_spans * NS + pair_idx[(s, i)]
                    rel = ch["col"] - i_col0[(s, i)]
                    ts_cur = rep * NT + t
                    mm = t_.matmul(
                        out=gp[:, ts_cur % GPSLOTS, i, :],
                        lhsT=stg[:, gpair % STGRING, rel, :],
                        rhs=sring[:, idx % SRING, :],
                        start=ch["start"], stop=ch["stop"],
                    )
                    mm.then_inc(ssf, 1)
                    idx += 1
                    if ch["last_of_pair"]:
                        t_.drain().then_inc(sstg, 1)
                    if idx - rep * n_chunks == tile_chunk_end[t]:
                        t_.drain().then_inc(st, 1)
                        pend.append(tile_seq)
                        tile_seq += 1
                while pend:
                    phase2(pend.pop(0))

        @block.scalar
        def _(a: bass.BassScalarEngine):
            tile_seq = 0
            for rep in range(reps):
                for t in range(NT):
                    a.wait_ge(st, tile_seq + 1)
                    if tile_seq >= 2:
                        a.wait_ge(sp2, tile_seq - 1)
                    a.activation(out=gsb[:, tile_seq % 2, :, :],
                                 in_=gp[:, tile_seq % GPSLOTS, :, :],
                                 func=mybir.ActivationFunctionType.Copy
                                 ).then_inc(sgp, 1)
                    a.wait_ge(sp2, tile_seq + 1)
                    if tile_seq + 1 > OTSLOTS:
                        a.wait_ge(sof, 16 * (tile_seq + 1 - OTSLOTS))
                    a.activation(out=ot[:, tile_seq % OTSLOTS, :],
                                 in_=op[:, tile_seq % OTSLOTS, :],
                                 func=mybir.ActivationFunctionType.Copy
                                 ).then_inc(sot, 1)
                    tile_seq += 1

    nc.compile()
    return nc


def _make_in_maps(cfg, prep, x, w_list):
    n_nodes = x.shape[0]
    x_pad = np.zeros((cfg.x_rows_pad, P), np.float32)
    x_pad[:n_nodes] = np.asarray(x, np.float32)
    x_pad = x_pad.astype(ml_dtypes.bfloat16)
    iota = np.tile(np.arange(DT, dtype=np.float32)[None, :], (P, 1)).astype(
        ml_dtypes.bfloat16)
    w_cat = np.concatenate([np.asarray(w, np.float32) for w in w_list], axis=0)

    return [{
        "x": x_pad,
        "idx": prep["idx_mats"][c],
        "dstf": prep["dstf_mats"][c],
        "iota": iota,
        "w": w_cat,
    } for c in range(N_CORES)]


def kernel(hidden_states, edges_i, edges_ii, edges_iii, edges_a,
           W_i, W_ii, W_iii, W_a):
    x = np.asarray(hidden_states, np.float32)
    cfg = _Cfg(x.shape[0])
    edges_list = [np.asarray(e) for e in (edges_i, edges_ii, edges_iii, edges_a)]
    w_list = [W_i, W_ii, W_iii, W_a]

    prep = _host_prep(cfg, edges_list)
    nc = _build_kernel(cfg, prep)
    in_maps = _make_in_maps(cfg, prep, x, w_list)

    res = run_bass_kernel_spmd(nc, in_maps, core_ids=list(range(N_CORES)))
    out = np.concatenate([res.results[c]["out"] for c in range(N_CORES)], axis=0)
    return out.astype(np.float32)
